# revision 14
# baseline (speedup 1.0000x reference)
"""Trainium2 Bass kernel for Bidirectional Temporal Self Attention.

out = x * (g1+g2+g3) where each g_b = sigmoid(rank1-attention(conv1d(mean_CHW(x)))).

Sharding (v2, fused single-sweep): shard x on H (8 rows per core) across all 8
cores, so every core holds a 5.4 MB slice of each batch item. Each item is
loaded once, stays resident in SBUF while a 120-byte AllReduce combines the
8 cores' partial (C,h-slice,W) sums into the full per-(n,t) sums, then the
resident tiles are scaled in place and stored. HBM traffic drops from ~245 MB
(load-twice baseline) to the 173 MB floor per core.

The tiny per-branch convs are folded into host-precomputed banded matrices
B[s,t] = w[s-t+p]/(C*H*W) (one [30,270] input), so conv+attention run as
small Tensor-engine matmuls; sigmoid is computed via the EXP table only
(1/(1+exp(-x))) so the Scalar engine never reloads activation tables.

Pipeline: items are emitted with a 2-item software lag (phase B + scale +
store of item i emitted after loads+reduces of item i+2), so the per-item
collective latency (~16 us) hides under the ~27 us/item DMA cadence and no
engine queue stalls on it.
"""
import numpy as np

import concourse.bass as bass
from concourse import bacc
import concourse.tile as tile
from concourse import mybir
from concourse import bass_utils

N, C, T, H, W = 16, 128, 30, 64, 44
NCORES = 8
HSL = H // NCORES          # 8 h-rows per core
SL = HSL * W               # 352 spatial elements per (c, t)
TH = 15                    # t-half: chunk size for loads/multiplies/stores
DVE_T = 10                 # t's scaled per-t on DVE; the rest broadcast on Pool
M = C * H * W              # mean divisor
F32 = mybir.dt.float32
X_AX = mybir.AxisListType.X
MUL = mybir.AluOpType.mult
ADD = mybir.AluOpType.add

WSPECS = [("wq1", 3), ("wk1", 3), ("wv1", 3),
          ("wq2", 5), ("wk2", 5), ("wv2", 5),
          ("wq3", 7), ("wk3", 7), ("wv3", 7)]

LAG = 2                    # software pipeline depth (items)


def _bcast(ap, n):
    """Append a stride-0 dim of size n to an AP (free-axis broadcast)."""
    return bass.AP(ap.tensor, ap.offset, list(ap.ap) + [[0, n]])


def build_band_matrices(ws: dict) -> np.ndarray:
    """Pack 9 banded conv matrices into [T, 9*T], scaled by 1/M.

    Column block 30*j holds matrix j (order = WSPECS order), with
    B[s, t] = w[s - t + p] / M for 0 <= s-t+p < k  (SAME cross-correlation:
    q[t] = sum_s y_sum[s] * B[s, t] == conv1d(y_sum/M, w)[t]).
    """
    out = np.zeros((T, 9 * T), dtype=np.float32)
    for j, (name, k) in enumerate(WSPECS):
        w = np.asarray(ws[name], dtype=np.float64).reshape(k)
        p = (k - 1) // 2
        B = np.zeros((T, T), dtype=np.float64)
        for t in range(T):
            for m in range(k):
                s = t + m - p
                if 0 <= s < T:
                    B[s, t] = w[m] / M
        out[:, 30 * j:30 * (j + 1)] = B.astype(np.float32)
    return out


def build_bass():
    nc = bacc.Bacc("TRN2", num_devices=NCORES)
    x = nc.declare_dram_parameter("x", [N, C, T, HSL, W], F32, isOutput=False)
    bmat = nc.declare_dram_parameter("B", [T, 9 * T], F32, isOutput=False)
    out = nc.declare_dram_parameter("out", [N, C, T, HSL, W], F32, isOutput=True)

    xvf = x[:].rearrange("n c t h w -> n c (t h w)")
    ovf = out[:].rearrange("n c t h w -> n c (t h w)")
    groups = [list(range(NCORES))]

    with tile.TileContext(nc) as tc:
        with (
            tc.tile_pool(name="data", bufs=9) as data_pool,
            tc.tile_pool(name="small", bufs=2) as small,
            tc.tile_pool(name="const", bufs=1) as const,
            tc.tile_pool(name="psS", bufs=2, space="PSUM") as psS,
            tc.tile_pool(name="psQKV", bufs=2, space="PSUM") as psQKV,
            tc.tile_pool(name="psP", bufs=1, space="PSUM") as psP,
            tc.tile_pool(name="psOp", bufs=2, space="PSUM") as psOp,
            tc.tile_pool(name="psC", bufs=1, space="PSUM") as psC,
            tc.tile_pool(name="dram", bufs=4, space="DRAM") as dram,
        ):
            # --- one-time constants ---
            B_sb = const.tile([T, 9 * T], F32, tag="B")
            nc.gpsimd.dma_start(B_sb[:], bmat[:])
            ones128 = const.tile([C, 1], F32, tag="ones128")
            nc.vector.memset(ones128[:], 1.0)
            ones_1xC = const.tile([1, C], F32, tag="ones_1xC")
            nc.vector.memset(ones_1xC[:], 1.0)

            state = {}

            def emit_load_reduce(n):
                P_n = small.tile([C, T], F32, tag="P")
                halves = []
                for ci in range(2):
                    tl = data_pool.tile([C, TH, SL], F32, tag="data")
                    nc.sync.dma_start(
                        tl[:].rearrange("p a b -> p (a b)"),
                        xvf[n, :, ci * TH * SL:(ci + 1) * TH * SL])
                    nc.vector.reduce_sum(P_n[:, ci * TH:(ci + 1) * TH], tl[:],
                                         axis=X_AX)
                    halves.append(tl)
                # partial sum over channels -> [1, T]
                ppart = psP.tile([1, T], F32, tag="ppart")
                nc.tensor.matmul(ppart[:], lhsT=ones128[:], rhs=P_n[:],
                                 start=True, stop=True)
                stage = small.tile([1, T], F32, tag="stage")
                nc.vector.tensor_copy(stage[:], ppart[:])
                cin = dram.tile([1, T], F32, tag="cin")
                cout = dram.tile([T, 1], F32, tag="cout")
                nc.gpsimd.dma_start(cin[:], stage[:])
                nc.gpsimd.collective_compute(
                    "AllReduce", ADD, replica_groups=groups,
                    ins=[cin.opt()], outs=[cout.opt()])
                state[n] = (halves, cout)

            def emit_phaseb_scale_store(n):
                halves, cout = state.pop(n)
                y_col = small.tile([T, 1], F32, tag="ycol")
                # scalar ring: waits only on this item's (long-done) CC
                nc.scalar.dma_start(y_col[:], cout[:])

                gsum = None
                for b in range(3):
                    qkv = psQKV.tile([T, 63], F32, tag="qkv")
                    qp = qkv[0:1, 0:T]
                    kp = qkv[0:1, 32:32 + T]
                    vp = qkv[0:T, 62:63]
                    nc.tensor.matmul(qp, lhsT=y_col[:],
                                     rhs=B_sb[:, 90 * b:90 * b + 30],
                                     start=True, stop=True)
                    nc.tensor.matmul(kp, lhsT=y_col[:],
                                     rhs=B_sb[:, 90 * b + 30:90 * b + 60],
                                     start=True, stop=True)
                    nc.tensor.matmul(vp, lhsT=B_sb[:, 90 * b + 60:90 * b + 90],
                                     rhs=y_col[:], start=True, stop=True)
                    q_sb = small.tile([1, T], F32, tag="q_sb")
                    k_sb = small.tile([1, T], F32, tag="k_sb")
                    nc.vector.tensor_copy(q_sb[:], qp)
                    nc.vector.tensor_copy(k_sb[:], kp)
                    S = psS.tile([T, T], F32, tag="S")
                    nc.tensor.matmul(S[:], lhsT=q_sb[:], rhs=k_sb[:],
                                     start=True, stop=True)
                    mx = small.tile([T, 1], F32, tag="mx")
                    nc.vector.reduce_max(mx[:], S[:], axis=X_AX)
                    nmx = small.tile([T, 1], F32, tag="nmx")
                    nc.vector.tensor_scalar_mul(nmx[:], mx[:], -1.0)
                    E = small.tile([T, T], F32, tag="E")
                    nc.scalar.activation(E[:], S[:],
                                         mybir.ActivationFunctionType.Exp,
                                         bias=nmx[:], scale=1.0)
                    Z = small.tile([T, 1], F32, tag="Z")
                    nc.vector.reduce_sum(Z[:], E[:], axis=X_AX)
                    R = small.tile([T, 1], F32, tag="R")
                    nc.vector.reciprocal(R[:], Z[:])
                    c_sb = small.tile([T, 1], F32, tag="c_sb")
                    nc.vector.tensor_mul(c_sb[:], vp, R[:])
                    op = psOp.tile([1, T], F32, tag="op")
                    nc.tensor.matmul(op[:], lhsT=c_sb[:], rhs=E[:],
                                     start=True, stop=True)
                    # sigmoid via exp: 1/(1+exp(-x)) -- EXP table only
                    eb = small.tile([1, T], F32, tag="eb")
                    nc.scalar.activation(eb[:], op[:],
                                         mybir.ActivationFunctionType.Exp,
                                         scale=-1.0)
                    nc.vector.tensor_scalar_add(eb[:], eb[:], 1.0)
                    if b == 0:
                        gsum = small.tile([1, T], F32, tag="gsum")
                        nc.vector.reciprocal(gsum[:], eb[:])
                    else:
                        nc.vector.reciprocal(eb[:], eb[:])
                        nc.vector.tensor_add(gsum[:], gsum[:], eb[:])
                scp = psC.tile([C, T], F32, tag="scp")
                nc.tensor.matmul(scp[:], lhsT=ones_1xC[:], rhs=gsum[:],
                                 start=True, stop=True)
                scales = small.tile([C, T], F32, tag="scales")
                nc.vector.tensor_copy(scales[:], scp[:])
                # scale multiply: first DVE_T t's as per-t tensor_scalar on
                # DVE (fast path), the rest as broadcast tensor_tensor on Pool
                for ci in range(2):
                    tl = halves[ci]
                    t0 = ci * TH
                    for tt in range(min(max(DVE_T - t0, 0), TH)):
                        nc.vector.tensor_scalar_mul(
                            tl[:, tt, :], tl[:, tt, :],
                            scales[:, t0 + tt:t0 + tt + 1])
                    if t0 + TH > DVE_T:
                        lo = max(0, DVE_T - t0)
                        nc.gpsimd.tensor_tensor(
                            out=tl[:, lo:TH, :], in0=tl[:, lo:TH, :],
                            in1=_bcast(scales[:, t0 + lo:t0 + TH], SL),
                            op=MUL)
                    nc.scalar.dma_start(
                        ovf[n, :, ci * TH * SL:(ci + 1) * TH * SL],
                        tl[:].rearrange("p a b -> p (a b)"))

            for i in range(N + LAG):
                if i >= LAG:
                    emit_phaseb_scale_store(i - LAG)
                if i < N:
                    emit_load_reduce(i)

    nc.compile()
    return nc


_NC_CACHE = None


def _get_nc():
    global _NC_CACHE
    if _NC_CACHE is None:
        _NC_CACHE = build_bass()
    return _NC_CACHE


def run(inputs, trace=False, **kw):
    nc = _get_nc()
    x = np.ascontiguousarray(inputs["x"], dtype=np.float32)
    assert x.shape == (N, C, T, H, W), x.shape
    bmat = build_band_matrices(inputs)
    in_maps = []
    for c in range(NCORES):
        m = {"x": np.ascontiguousarray(x[:, :, :, HSL * c:HSL * (c + 1), :]),
             "B": bmat}
        in_maps.append(m)
    res = bass_utils.run_bass_kernel_spmd(
        nc, in_maps, core_ids=list(range(NCORES)), trace=trace, **kw)
    outs = np.concatenate([r["out"] for r in res.results], axis=3)
    return outs, res


def kernel(**inputs) -> np.ndarray:
    outs, _ = run(inputs, trace=False)
    return outs


# revision 15
# speedup vs baseline: 1.0716x; 1.0716x over previous
"""Trainium2 Bass kernel for Bidirectional Temporal Self Attention.

out = x * (g1+g2+g3) where each g_b = sigmoid(rank1-attention(conv1d(mean_CHW(x)))).

Sharding (v2, fused single-sweep): shard x on H (8 rows per core) across all 8
cores, so every core holds a 5.4 MB slice of each batch item. Each item is
loaded once, stays resident in SBUF while a 120-byte AllReduce combines the
8 cores' partial (C,h-slice,W) sums into the full per-(n,t) sums, then the
resident tiles are scaled in place and stored. HBM traffic drops from ~245 MB
(load-twice baseline) to the 173 MB floor per core.

The tiny per-branch convs are folded into host-precomputed banded matrices
B[s,t] = w[s-t+p]/(C*H*W) (one [30,270] input), so conv+attention run as
small Tensor-engine matmuls; sigmoid is computed via the EXP table only
(1/(1+exp(-x))) so the Scalar engine never reloads activation tables.

Pipeline: items are emitted with a 2-item software lag (phase B + scale +
store of item i emitted after loads+reduces of item i+2), so the per-item
collective latency (~16 us) hides under the ~27 us/item DMA cadence and no
engine queue stalls on it.
"""
import numpy as np

import concourse.bass as bass
from concourse import bacc
import concourse.tile as tile
from concourse import mybir
from concourse import bass_utils

N, C, T, H, W = 16, 128, 30, 64, 44
NCORES = 8
HSL = H // NCORES          # 8 h-rows per core
SL = HSL * W               # 352 spatial elements per (c, t)
TH = 15                    # t-half: chunk size for loads/multiplies/stores
DVE_T = 10                 # t's scaled per-t on DVE; the rest broadcast on Pool
M = C * H * W              # mean divisor
F32 = mybir.dt.float32
X_AX = mybir.AxisListType.X
MUL = mybir.AluOpType.mult
ADD = mybir.AluOpType.add

WSPECS = [("wq1", 3), ("wk1", 3), ("wv1", 3),
          ("wq2", 5), ("wk2", 5), ("wv2", 5),
          ("wq3", 7), ("wk3", 7), ("wv3", 7)]

LAG = 2                    # software pipeline depth (items)


def _bcast(ap, n):
    """Append a stride-0 dim of size n to an AP (free-axis broadcast)."""
    return bass.AP(ap.tensor, ap.offset, list(ap.ap) + [[0, n]])


def build_band_matrices(ws: dict) -> np.ndarray:
    """Pack 9 banded conv matrices into [T, 9*T], scaled by 1/M.

    Column block 30*j holds matrix j (order = WSPECS order), with
    B[s, t] = w[s - t + p] / M for 0 <= s-t+p < k  (SAME cross-correlation:
    q[t] = sum_s y_sum[s] * B[s, t] == conv1d(y_sum/M, w)[t]).
    """
    out = np.zeros((T, 9 * T), dtype=np.float32)
    for j, (name, k) in enumerate(WSPECS):
        w = np.asarray(ws[name], dtype=np.float64).reshape(k)
        p = (k - 1) // 2
        B = np.zeros((T, T), dtype=np.float64)
        for t in range(T):
            for m in range(k):
                s = t + m - p
                if 0 <= s < T:
                    B[s, t] = w[m] / M
        out[:, 30 * j:30 * (j + 1)] = B.astype(np.float32)
    return out


def build_bass():
    nc = bacc.Bacc("TRN2", num_devices=NCORES)
    x = nc.declare_dram_parameter("x", [N, C, T, HSL, W], F32, isOutput=False)
    bmat = nc.declare_dram_parameter("B", [T, 9 * T], F32, isOutput=False)
    out = nc.declare_dram_parameter("out", [N, C, T, HSL, W], F32, isOutput=True)

    xvf = x[:].rearrange("n c t h w -> n c (t h w)")
    ovf = out[:].rearrange("n c t h w -> n c (t h w)")
    groups = [list(range(NCORES))]

    with tile.TileContext(nc) as tc:
        with (
            tc.tile_pool(name="data", bufs=9) as data_pool,
            tc.tile_pool(name="small", bufs=2) as small,
            tc.tile_pool(name="const", bufs=1) as const,
            tc.tile_pool(name="psS", bufs=2, space="PSUM") as psS,
            tc.tile_pool(name="psQKV", bufs=2, space="PSUM") as psQKV,
            tc.tile_pool(name="psP", bufs=1, space="PSUM") as psP,
            tc.tile_pool(name="psOp", bufs=2, space="PSUM") as psOp,
            tc.tile_pool(name="psC", bufs=1, space="PSUM") as psC,
            tc.tile_pool(name="dram", bufs=4, space="DRAM") as dram,
        ):
            # --- one-time constants ---
            B_sb = const.tile([T, 9 * T], F32, tag="B")
            nc.gpsimd.dma_start(B_sb[:], bmat[:])
            ones128 = const.tile([C, 1], F32, tag="ones128")
            nc.vector.memset(ones128[:], 1.0)
            ones_1xC = const.tile([1, C], F32, tag="ones_1xC")
            nc.vector.memset(ones_1xC[:], 1.0)

            state = {}

            def emit_load_reduce(n):
                P_n = small.tile([C, T], F32, tag="P")
                halves = []
                for ci in range(2):
                    tl = data_pool.tile([C, TH, SL], F32, tag="data")
                    nc.sync.dma_start(
                        tl[:].rearrange("p a b -> p (a b)"),
                        xvf[n, :, ci * TH * SL:(ci + 1) * TH * SL])
                    if ci == 0:
                        # first half: free-axis reduce on DVE
                        nc.vector.reduce_sum(P_n[:, 0:TH], tl[:], axis=X_AX)
                    else:
                        # second half: per-t accumulate on the Scalar engine
                        # (own datapath -- halves the DVE reduce load)
                        scr = small.tile([C, SL], F32, tag="scr")
                        for tt in range(TH):
                            nc.scalar.activation(
                                scr[:], tl[:, tt, :],
                                mybir.ActivationFunctionType.Copy,
                                accum_out=P_n[:, TH + tt:TH + tt + 1])
                    halves.append(tl)
                # partial sum over channels -> [1, T]
                ppart = psP.tile([1, T], F32, tag="ppart")
                nc.tensor.matmul(ppart[:], lhsT=ones128[:], rhs=P_n[:],
                                 start=True, stop=True)
                stage = small.tile([1, T], F32, tag="stage")
                nc.vector.tensor_copy(stage[:], ppart[:])
                cin = dram.tile([1, T], F32, tag="cin")
                cout = dram.tile([T, 1], F32, tag="cout")
                nc.gpsimd.dma_start(cin[:], stage[:])
                nc.gpsimd.collective_compute(
                    "AllReduce", ADD, replica_groups=groups,
                    ins=[cin.opt()], outs=[cout.opt()])
                state[n] = (halves, cout)

            def emit_phaseb_scale_store(n):
                halves, cout = state.pop(n)
                y_col = small.tile([T, 1], F32, tag="ycol")
                # scalar ring: waits only on this item's (long-done) CC
                nc.scalar.dma_start(y_col[:], cout[:])

                gsum = None
                for b in range(3):
                    qkv = psQKV.tile([T, 63], F32, tag="qkv")
                    qp = qkv[0:1, 0:T]
                    kp = qkv[0:1, 32:32 + T]
                    vp = qkv[0:T, 62:63]
                    nc.tensor.matmul(qp, lhsT=y_col[:],
                                     rhs=B_sb[:, 90 * b:90 * b + 30],
                                     start=True, stop=True)
                    nc.tensor.matmul(kp, lhsT=y_col[:],
                                     rhs=B_sb[:, 90 * b + 30:90 * b + 60],
                                     start=True, stop=True)
                    nc.tensor.matmul(vp, lhsT=B_sb[:, 90 * b + 60:90 * b + 90],
                                     rhs=y_col[:], start=True, stop=True)
                    q_sb = small.tile([1, T], F32, tag="q_sb")
                    k_sb = small.tile([1, T], F32, tag="k_sb")
                    nc.vector.tensor_copy(q_sb[:], qp)
                    nc.vector.tensor_copy(k_sb[:], kp)
                    S = psS.tile([T, T], F32, tag="S")
                    nc.tensor.matmul(S[:], lhsT=q_sb[:], rhs=k_sb[:],
                                     start=True, stop=True)
                    E = small.tile([T, T], F32, tag="E")
                    nc.scalar.activation(E[:], S[:],
                                         mybir.ActivationFunctionType.Exp)
                    Z = small.tile([T, 1], F32, tag="Z")
                    nc.vector.reduce_sum(Z[:], E[:], axis=X_AX)
                    R = small.tile([T, 1], F32, tag="R")
                    nc.vector.reciprocal(R[:], Z[:])
                    c_sb = small.tile([T, 1], F32, tag="c_sb")
                    nc.vector.tensor_mul(c_sb[:], vp, R[:])
                    op = psOp.tile([1, T], F32, tag="op")
                    nc.tensor.matmul(op[:], lhsT=c_sb[:], rhs=E[:],
                                     start=True, stop=True)
                    # sigmoid via exp: 1/(1+exp(-x)) -- EXP table only
                    eb = small.tile([1, T], F32, tag="eb")
                    nc.scalar.activation(eb[:], op[:],
                                         mybir.ActivationFunctionType.Exp,
                                         scale=-1.0)
                    nc.vector.tensor_scalar_add(eb[:], eb[:], 1.0)
                    if b == 0:
                        gsum = small.tile([1, T], F32, tag="gsum")
                        nc.vector.reciprocal(gsum[:], eb[:])
                    else:
                        nc.vector.reciprocal(eb[:], eb[:])
                        nc.vector.tensor_add(gsum[:], gsum[:], eb[:])
                scp = psC.tile([C, T], F32, tag="scp")
                nc.tensor.matmul(scp[:], lhsT=ones_1xC[:], rhs=gsum[:],
                                 start=True, stop=True)
                scales = small.tile([C, T], F32, tag="scales")
                nc.vector.tensor_copy(scales[:], scp[:])
                for ci in range(2):
                    tl = halves[ci]
                    t0 = ci * TH
                    for tt in range(TH):
                        nc.vector.tensor_scalar_mul(
                            tl[:, tt, :], tl[:, tt, :],
                            scales[:, t0 + tt:t0 + tt + 1])
                    nc.gpsimd.dma_start(
                        ovf[n, :, ci * TH * SL:(ci + 1) * TH * SL],
                        tl[:].rearrange("p a b -> p (a b)"))

            for i in range(N + LAG):
                if i >= LAG:
                    emit_phaseb_scale_store(i - LAG)
                if i < N:
                    emit_load_reduce(i)

    nc.compile()
    return nc


_NC_CACHE = None


def _get_nc():
    global _NC_CACHE
    if _NC_CACHE is None:
        _NC_CACHE = build_bass()
    return _NC_CACHE


def run(inputs, trace=False, **kw):
    nc = _get_nc()
    x = np.ascontiguousarray(inputs["x"], dtype=np.float32)
    assert x.shape == (N, C, T, H, W), x.shape
    bmat = build_band_matrices(inputs)
    in_maps = []
    for c in range(NCORES):
        m = {"x": np.ascontiguousarray(x[:, :, :, HSL * c:HSL * (c + 1), :]),
             "B": bmat}
        in_maps.append(m)
    res = bass_utils.run_bass_kernel_spmd(
        nc, in_maps, core_ids=list(range(NCORES)), trace=trace, **kw)
    outs = np.concatenate([r["out"] for r in res.results], axis=3)
    return outs, res


def kernel(**inputs) -> np.ndarray:
    outs, _ = run(inputs, trace=False)
    return outs
